# revision 31
# baseline (speedup 1.0000x reference)
"""Conv2d 3x3 (im2col GEMM) on 8 TRN2 NeuronCores.

Problem: x[16,64,112,112] (*) w[576,64] + b[64] -> out[16,64,112,112]
(3x3, stride 1, pad 1, NCHW, im2col patch order (c, kh, kw)).

Strategy
--------
Data-parallel over batch: 2 images per core, 8 cores, no collectives.

Per image, an implicit-GEMM formulation needing 3 full-width bf16
matmuls per (4-row x 112-col) output chunk:

  * The host pre-packs each image into z[128, F+1] bf16: partitions
    0:64 hold the image flattened with ROW STRIDE 113 (one zero
    between consecutive rows) plus a 114-elem lead pad ("zt");
    partitions 64:128 hold the same stream shifted LEFT by one
    element ("zb"). The inter-row zeros make the kw edge taps read
    exact padding (no edge corrections), and baking the shifted copy
    host-side makes each z segment ONE aligned [128, cols] DMA --
    per-DMA enqueue cost (~0.65us sequencer + ~0.63us shared HWDGE
    slot) dominates at this size, so fewer/bigger DMAs win.
  * For each kh in {0,1,2} one matmul with the block lhsT
        [[w(kh,1), w(kh,0)],
         [w(kh,2),    0   ]]
    over rhs z[:, 113*(4c+kh) : +454] accumulates chunk c's psum
    [128, 454]:
      psum[0:64,  q] += w1*x(y, q-1) + w2*x(y, q)   (zt/zb halves)
      psum[64:128,q] += w0*x(y, q-1)                (zt half)
    Chunks run in groups of 4 (two 2-bank [128, 1024] chunk-pair psum
    tiles) with kh as the middle loop so the PE switches stationary
    weights 3 times per chunk-pair, not 6. (A matching argument over
    the 9 taps x consecutive-kw slot structure shows ~336 matmul-
    widths/image is optimal for this ISA, so PE work is at its floor;
    the schedule below keeps the PE gapless.)
  * Per chunk-pair fold (PSUM allows only one PSUM operand per
    instruction, so two stages):
      ACT: tb[64,k,r,j] = psum[64:128, 512k + 113r + j] + bias
      DVE: og[...]      = psum[0:64,  512k + 113r + j + 1] + tb
    og is a compact bf16 [64, 1792] per-group tile.

Schedule (the HW-visible wins over a naive emission):
  * Startup: a DVE-memset zero tile feeds 8 dummy matmuls that open
    the PE HAM warm-up window at ~1.4us with no load dependency,
    while lhsT (ACT HWDGE ring) and the first z slice (SP ring) load;
    the first real matmul issues ~4us in and the PE never idles
    again until its last matmul.
  * z segments ride the SP ring just-in-time (2 groups ahead).
  * og stores: the early kernel window must move BOTH images' z
    (6.6MB) plus og stores through ~358GB/s of HBM; the first
    image's g>=2 og stores are therefore deferred and emitted during
    the next image's groups (opool bufs=8 holds them), which keeps
    the crunch window under capacity and the PE stall-free.
  * Mid-stream og stores ride the gpsimd SWDGE queue, where their
    wait-for-fold blocks nothing; each image's LAST-group store and
    the final image's stores ride the HWDGE rings instead, whose
    ~0.6us completion beats SWDGE's ~2-4us drain at the kernel tail.
    Store enqueues never sit between compute ops on ACT/DVE: a
    waiting DMA enqueue blocks every later op on that sequencer's
    FIFO, which would re-serialize the fold pipeline.

All matmul data is bf16 (1 cycle/column on the PE, half the DMA bytes
of fp32); psum accumulation is fp32. The output is stored bf16 and
upcast to fp32 on the host (max rel err ~4e-3, well inside the 2e-2
gate). fp8 (DoubleRow) would halve PE time but its 3-bit mantissa
puts the dot-product error at ~4e-2: fails the gate.

TimelineSim one-shot: 41.8us vs 52.6us for the pre-session baseline
(For_i-amortized HW steady state: ~52us/iter vs ~54; the hardware
loop's all-engine barrier serializes iteration edges, so the one-shot
startup/tail wins mostly show there).
"""

import contextlib

import numpy as np

import concourse.bacc as bacc
import concourse.mybir as mybir
import concourse.tile as tile
from concourse import bass_utils

try:
    from ml_dtypes import bfloat16 as np_bf16
except ImportError:  # jax ships ml_dtypes; fall back just in case
    import jax.numpy as _jnp

    np_bf16 = _jnp.bfloat16

# problem geometry (hardcoded per contract)
B, CIN, H, W = 16, 64, 112, 112
COUT = 64
NCORES = 8
IMGS = B // NCORES  # images per core

S = 113                        # z row stride (W + 1 zero gap)
OFF = S + 1                    # lead pad: one padded row + 1 left-tap elem
ROWS_PER_CHUNK = 4
MMW = ROWS_PER_CHUNK * S + 2   # 454: matmul moving width (<=512 psum bank)
NCHUNK = H // ROWS_PER_CHUNK   # 28
GROUP = 4                      # chunks per weight-switch group / og store
NGROUP = NCHUNK // GROUP       # 7
F = S * (4 * (NCHUNK - 1) + 2) + MMW  # 12884: max rhs slice end
HWC = H * W                    # 12544 outputs per image per channel
OGW = GROUP * ROWS_PER_CHUNK * W      # 1792 outputs per og group tile

f32 = mybir.dt.float32
bf16 = mybir.dt.bfloat16

_cache = {}


def _prep_inputs(x):
    """Host: pack x -> z[B, 128, F+1] bf16: partitions 0:64 hold the
    113-stride zt stream, 64:128 the same shifted left by one, so each
    kernel segment is ONE aligned [128, cols] DMA."""
    xb = x.astype(np_bf16)
    z = np.zeros((B, 128, F + 1), np_bf16)
    body = z[:, 0:64, OFF: OFF + H * S].reshape(B, CIN, H, S)
    body[:, :, :, 0:W] = xb
    z[:, 64:128, 0:F] = z[:, 0:64, 1: F + 1]
    return z


def _prep_weights(weight):
    """Host: block lhsT matrices packed [128, 3*128] bf16 (kh-major)."""
    w = np.asarray(weight, np.float32).reshape(CIN, 3, 3, COUT)
    lt = np.zeros((128, 3, 128), np.float32)
    for kh in range(3):
        lt[0:64, kh, 0:64] = w[:, kh, 1, :]
        lt[0:64, kh, 64:128] = w[:, kh, 0, :]
        lt[64:128, kh, 0:64] = w[:, kh, 2, :]
    return lt.reshape(128, 384).astype(np_bf16)


# Chunk-group g needs z columns [0, S*(16g+14)+MMW).
_SEG_END = [min(S * (16 * g + 14) + MMW, F) for g in range(NGROUP)]


def _build(repeat=None, unroll=1):
    """repeat: For_i hardware loop count (timing). unroll: straight-line
    duplication of the image pipeline (TimelineSim steady-state probe)."""
    nc = bacc.Bacc("TRN2", target_bir_lowering=False, debug=False,
                   num_devices=NCORES)

    z_d = nc.dram_tensor("z", (IMGS, 128, F + 1), bf16,
                         kind="ExternalInput")
    lt_d = nc.dram_tensor("lhsT", (128, 384), bf16, kind="ExternalInput")
    b_d = nc.dram_tensor("bias", (COUT,), f32, kind="ExternalInput")
    o_d = nc.dram_tensor("out", (IMGS, COUT, H, W), bf16,
                         kind="ExternalOutput")

    ov = o_d.ap().rearrange("b c h w -> b c (h w)")

    def pair_view(ps, half, shift):
        """[64, k=2, r=4, j=112] view of a chunk-pair psum tile."""
        return (
            ps[64 * half: 64 * half + 64, :]
            .rearrange("q (k c) -> q k c", c=512)
            [:, :, shift: shift + 4 * S]
            .rearrange("q k (r j) -> q k r j", j=S)
            [:, :, :, 0:W]
        )

    with tile.TileContext(nc) as tc:
        with (
            tc.tile_pool(name="wpool", bufs=1) as wpool,
            tc.tile_pool(name="zpool", bufs=2) as zpool,
            tc.tile_pool(name="opool", bufs=8) as opool,
            tc.tile_pool(name="tpool", bufs=4) as tpool,
            tc.tile_pool(name="ppool", bufs=4, space="PSUM") as ppool,
        ):
            # --- warm-up seed: zero tile on DVE, no load dependency ---
            wd = wpool.tile([128, 128], bf16, name="wd", tag="wd")
            nc.vector.memset(wd[:, :], 0.0)

            # --- weights staging: ACT HWDGE ring, in parallel with the
            # z head on the SP ring; bias is emitted later (after the z
            # head) since it is first needed by stage-1 ---
            ltt = wpool.tile([128, 384], bf16, name="lhsT", tag="lhsT")
            nc.scalar.dma_start(ltt[:, :], lt_d.ap())
            bias = wpool.tile([COUT, 1], f32)
            lhs = [ltt[:, 128 * kh: 128 * (kh + 1)] for kh in range(3)]

            def warm_pe(n, width=128):
                """Dummy matmuls on the zero tile: open the PE HAM
                warm-up window while weights and z stream in."""
                wps = ppool.tile([128, 1024], f32, name="warm", tag="ps")
                for _ in range(n):
                    nc.tensor.matmul(wps[:, 0:width], wd[:, 0:128],
                                     wd[:, 0:width], start=True, stop=True)

            loop_cm = (
                tc.For_i(0, repeat, 1)
                if repeat is not None else contextlib.nullcontext()
            )
            with loop_cm:
                imgs = [i % IMGS for i in range(IMGS * unroll)]
                pending = []    # previous image's last og store (SP ring)
                deferred = []   # previous image's g2+ og stores (Pool)
                for pos, img in enumerate(imgs):
                    first, last = pos == 0, pos == len(imgs) - 1
                    z = zpool.tile([128, F + 1], bf16, name="z", tag="z")

                    cursor = [0]

                    def load_to(e):
                        """One aligned [128, cols] segment (SP ring)."""
                        cur = cursor[0]
                        nc.sync.dma_start(z[:, cur:e],
                                          z_d.ap()[img][:, cur:e])
                        cursor[0] = e

                    def load_seg(g):
                        load_to(_SEG_END[g])

                    pair0_end = S * 6 + MMW
                    load_to(pair0_end)
                    if first:
                        warm_pe(8)
                    load_to(_SEG_END[0])
                    load_seg(1)
                    if first:
                        nc.sync.dma_start(
                            bias[:, :],
                            b_d.ap().rearrange("(c one) -> c one", one=1))
                    for g in range(NGROUP):
                        if g + 2 < NGROUP:
                            load_seg(g + 2)
                        if deferred:
                            # previous image's deferred og store: its DMA
                            # now runs AFTER this image's z prefetches,
                            # keeping the early-kernel HBM window (both
                            # images' z + first og stores) under capacity
                            nc.gpsimd.dma_start(*deferred.pop(0))
                        og = opool.tile([COUT, OGW], bf16, name="og",
                                        tag="og")
                        final = last and g == NGROUP - 1
                        for p in range(2):
                            ps = ppool.tile([128, 1024], f32, name="ps",
                                            tag="ps")
                            for kh in range(3):
                                for k in range(2):
                                    c = g * GROUP + p * 2 + k
                                    a = S * (4 * c + kh)
                                    nc.tensor.matmul(
                                        ps[:, 512 * k: 512 * k + MMW],
                                        lhs[kh],
                                        z[:, a: a + MMW],
                                        start=(kh == 0),
                                        stop=(kh == 2),
                                    )
                            tb = tpool.tile([COUT, 896], bf16, name="tb",
                                            tag="tb")
                            tbv = tb[:, :].rearrange(
                                "q (k r j) -> q k r j", r=4, j=W)
                            nc.scalar.add(tbv, pair_view(ps, 1, 0),
                                          bias[:, :])
                            ogv = og[:, p * 896: (p + 1) * 896].rearrange(
                                "q (k r j) -> q k r j", r=4, j=W)
                            nc.vector.tensor_add(ogv, pair_view(ps, 0, 1),
                                                 tbv)
                            if p == 1 and pending:
                                # previous image's last og store (its fold
                                # is long done; SP ring is past this
                                # image's head loads)
                                for dst_, src_ in pending:
                                    nc.sync.dma_start(dst_, src_)
                                pending = []
                            if final:
                                nc.sync.dma_start(
                                    ov[img][:, g * OGW + p * 896:
                                            g * OGW + (p + 1) * 896],
                                    og[:, p * 896: (p + 1) * 896])
                        if g == NGROUP - 1 and not final:
                            # each image's last-group store rides the SP
                            # ring, flushed during the next image's first
                            # group (past its head loads)
                            pending.append(
                                (ov[img][:, g * OGW: (g + 1) * OGW],
                                 og[:, :]))
                        elif not final:
                            if g >= 2 and not last:
                                # defer past the early-kernel HBM crunch
                                deferred.append(
                                    (ov[img][:, g * OGW: (g + 1) * OGW],
                                     og[:, :]))
                            else:
                                nc.gpsimd.dma_start(
                                    ov[img][:, g * OGW: (g + 1) * OGW],
                                    og[:, :])

    nc.compile()
    return nc


def kernel(x: np.ndarray, weight: np.ndarray, bias: np.ndarray,
           **_ignored) -> np.ndarray:
    if "nc" not in _cache:
        _cache["nc"] = _build()
    nc = _cache["nc"]

    z_np = _prep_inputs(np.asarray(x, np.float32).reshape(B, CIN, H, W))
    lt_np = _prep_weights(weight)
    b_np = np.ascontiguousarray(bias, dtype=np.float32)

    in_maps = [
        {
            "z": np.ascontiguousarray(z_np[i * IMGS: (i + 1) * IMGS]),
            "lhsT": lt_np,
            "bias": b_np,
        }
        for i in range(NCORES)
    ]
    res = bass_utils.run_bass_kernel_spmd(
        nc, in_maps, core_ids=list(range(NCORES)))
    out = np.concatenate([np.asarray(r["out"]) for r in res.results], axis=0)
    return out.astype(np.float32).reshape(B, COUT, H, W)
